# revision 85
# baseline (speedup 1.0000x reference)
"""Trainium2 Bass kernel for a 3-modality grouped BertSelfAttention (v3).

Problem (hardcoded shapes):
  B=4, S=2048, H=768, NH=12 heads of D=64, G=3 modality groups x E=4 heads.
  Group g's input is embeds{g+1}; heads [4g, 4g+4) attend over it.
  out[b, s, h*64:(h+1)*64] = softmax(Q_h K_h^T / 8) V_h  per (b, h).

Sharding (8 cores): core c handles batch b = c//2 and a half of the 12 heads
(6 heads). Halves are chosen so each core needs only 2 of the 3 embeds:
  half 0 -> heads [0,1,2,3, 4,5]   (embeds1 x4, embeds2 x2)
  half 1 -> heads [8,9,10,11, 6,7] (embeds3 x4, embeds2 x2)
Heads are processed in pairs (3 pairs/core); each pair shares one input.

v3 changes vs v2:
  - PV is flipped: P (=e_t tiles) is the stationary operand, v the moving
    one, so each matmul streams only n=65 rows (64 ctx + 1 denominator)
    instead of n=512 with a 65-row stationary. ctx comes out as [s, d]
    f32 in PSUM, is staged [128, 4, 65] per (pair, e, s-chunk) and DMA'd
    as one contiguous block. Softmax division stays on the host.
  - Exp tiles split ACT/DVE (Schraudolph bf16-bit exp on DVE); proj
    bias-add+cast evictions ride ACT's Identity+bias; v-ones memsets on
    GpSimd (which cannot touch PSUM on real HW).
  - Scores stay bf16: fp8 DoubleRow measures ~1.0 cy/row on hardware
    (not the modeled 0.5) and its 2x-wide ldweights streams regress the
    kernel, so DR is not used anywhere.  (LAM=16 weight pre-scale kept;
    the exp scale absorbs 1/LAM^2.)
"""

import sys

if "/opt/trn_rl_repo" not in sys.path:
    sys.path.insert(0, "/opt/trn_rl_repo")

import math

import ml_dtypes
import numpy as np

import concourse.bass as bass
import concourse.tile as tile
from concourse import bacc, mybir
from concourse.bass_utils import run_bass_kernel_spmd
from concourse.masks import make_identity

B, S, H, NH, D = 4, 2048, 768, 12, 64
SCALE = 1.0 / math.sqrt(D)
LAM = 16.0                      # host pre-scale on Wq/Wk/bq/bk
LW = 64.0                       # extra weight pre-scale so the fp8 hi/lo
                                # split of W stays out of e4m3 subnormals
QK_SCALE = SCALE / (LAM * LAM)  # exp input scale for fp8 scores
# Schraudolph constants: bf16 bits of exp(QK_SCALE*s) ~ round(s*EXP_A + EXP_B)
EXP_A = QK_SCALE * 128 * math.log2(math.e)
EXP_B = 127.0 * 128 - 5.5 + 0.5
HC = H // 128          # 6 contraction chunks of 128
NPAIR = 3              # head pairs per core
SC = 512               # s-chunk width
NSC = S // SC          # 4
NTT = S // 128         # 16 t-tiles
BF16 = mybir.dt.bfloat16
F32 = mybir.dt.float32
F8 = mybir.dt.float8e4
DR = mybir.MatmulPerfMode.DoubleRow
DVE_TPS = (1, 3, 5, 7)  # t-tile pairs whose exp runs as Schraudolph on DVE

_CACHE = {}


def _build_nc(reps=1):
    nc = bacc.Bacc("TRN2", target_bir_lowering=False, debug=False, num_devices=8)

    # Host-prepped, per-partition-contiguous layouts.
    xa = nc.dram_tensor("xa", [128, HC, S], BF16, kind="ExternalInput")
    xb = nc.dram_tensor("xb", [128, HC, S], BF16, kind="ExternalInput")
    # wqkv[p, kind(q,k,v), pair, hc, m]; m packs both heads (64+64).
    wqkv = nc.dram_tensor("wqkv", [128, 3, NPAIR, HC, 128], BF16, kind="ExternalInput")
    # bias[p, kind, pair]: per-d bias for both heads stacked on partitions.
    bias = nc.dram_tensor("bias", [128, 3, NPAIR], F32, kind="ExternalInput")
    # ctx^T+den blocks: out[head(2p+e), s%128, s//128, 64 ctx + 1 den].
    out = nc.dram_tensor("out", [NPAIR * 2, 128, NTT, 65], F32, kind="ExternalOutput")

    with tile.TileContext(nc) as tc:
        with (
            tc.tile_pool(name="consts", bufs=1) as consts,
            tc.tile_pool(name="xpool", bufs=1) as xpool,
            tc.tile_pool(name="qkpool", bufs=1) as qkpool,
            tc.tile_pool(name="vtpool", bufs=1) as vtpool,
            tc.tile_pool(name="vpool", bufs=1) as vpool,
            tc.tile_pool(name="epool", bufs=3) as epool,
            tc.tile_pool(name="cspool", bufs=2) as cspool,
            tc.tile_pool(name="st_psum", bufs=3, space="PSUM") as st_psum,
            tc.tile_pool(name="ctx_psum", bufs=1, space="PSUM") as ctx_psum,
            tc.tile_pool(name="aux_psum", bufs=1, space="PSUM") as aux_psum,
        ):
            ident = consts.tile([128, 128], BF16)
            make_identity(nc, ident)

            w_sb = consts.tile([128, 3, NPAIR, HC, 128], BF16)
            b_sb = consts.tile([128, 3, NPAIR], F32)
            # K-weights land first so the very first projection can start;
            # Q/V weights stream in behind the first xa chunk (see rep body).
            # pair-0 k-weights in their own (first) transfer: the opening
            # projection only needs those 98KB, not the full kind slab.
            nc.sync.dma_start(out=w_sb[:, 1, 0], in_=wqkv.ap()[:, 1, 0])
            nc.sync.dma_start(out=w_sb[:, 1, 1:], in_=wqkv.ap()[:, 1, 1:])
            nc.sync.dma_start(out=b_sb, in_=bias.ap())
            w_loaded = [False]

            def emit_w_load(kind):
                nc.sync.dma_start(out=w_sb[:, kind], in_=wqkv.ap()[:, kind])

            xa_sb = xpool.tile([128, HC, S], BF16, tag="x_xa", name="x_xa")
            xb_sb = xpool.tile([128, HC, S], BF16, tag="x_xb", name="x_xb")
            x_sb = [xa_sb, xb_sb]
            x_dram = [xa, xb]

            qt_sb = [
                qkpool.tile([128, S], BF16, tag=f"qt{p}", name=f"qt{p}")
                for p in range(NPAIR)
            ]
            kt_sb = [
                qkpool.tile([128, S], BF16, tag=f"kt{p}", name=f"kt{p}")
                for p in range(NPAIR)
            ]

            def emit_x_load(i, sc, hcs=slice(None)):
                # s-chunked loads: projections for s-chunk sc only need that
                # slice, so the first matmuls start after ~1/4 of the load.
                ssl = bass.ts(sc, SC)
                nc.sync.dma_start(
                    out=x_sb[i][:, hcs, ssl], in_=x_dram[i].ap()[:, hcs, ssl],
                )

            def _one_rep():
                v_sb = {}

                def emit_proj(p, sc, kind, dst):
                    # dst[:, ssl] (bf16) = W^T x + b
                    xs = x_sb[0] if p < 2 else x_sb[1]
                    ssl = bass.ts(sc, SC)
                    pq = aux_psum.tile([128, SC], F32, tag="aux", name="pproj")
                    for hc in range(HC):
                        nc.tensor.matmul(
                            pq,
                            w_sb[:, kind, p, hc, :],
                            xs[:, hc, ssl],
                            start=(hc == 0),
                            stop=(hc == HC - 1),
                        )
                    # bias-add + cast rides the ACT engine (Identity+bias is
                    # in the same act table as Exp, so no table reloads).
                    nc.scalar.add(
                        out=dst[:, ssl], in_=pq, add=b_sb[:, kind, p : p + 1])

                def emit_vones(p):
                    # ones columns 64 and 129 feed the softmax denominator rows
                    nc.gpsimd.memset(v_sb[p][:, :, 64:65], 1.0)
                    nc.gpsimd.memset(v_sb[p][:, :, 129:130], 1.0)

                def emit_vtrans(p, quarter, vt):
                    # PE-transpose 4 t-tiles of vt into v_sb's [V1 | 1 | V2 | 1]
                    # (an XBAR dma_start_transpose version passes CoreSim but
                    # produces NaN on hardware — do not resurrect it blindly)
                    ptp = aux_psum.tile([128, 4, 128], BF16, tag="aux", name="ptp")
                    for i in range(4):
                        tt = quarter * 4 + i
                        nc.tensor.transpose(
                            ptp[:, i, :], vt[:, bass.ts(tt, 128)], ident,
                        )
                    # dst columns {0..63} and {65..128}: view 130 as 2 blocks
                    # of 65 and take the first 64 of each.
                    nc.vector.tensor_copy(
                        v_sb[p][:, bass.ds(quarter * 4, 4), :].rearrange(
                            "p t (h d) -> p t h d", h=2
                        )[:, :, :, 0:64],
                        ptp.rearrange("p t (h d) -> p t h d", h=2),
                    )

                def proj_units(p, skip_first=False):
                    v_sb[p] = vpool.tile([128, NTT, 130], BF16, tag=f"v{p}", name=f"v{p}")
                    vt = vtpool.tile([128, S], BF16, tag="vt", name="vt")
                    units = []
                    for sc in range(NSC):
                        if skip_first and sc == 0:
                            continue
                        units.append(lambda p=p, sc=sc: emit_proj(p, sc, 1, kt_sb[p]))
                        units.append(lambda p=p, sc=sc: emit_proj(p, sc, 0, qt_sb[p]))
                    for sc in range(NSC):
                        units.append(lambda p=p, sc=sc, vt=vt: emit_proj(p, sc, 2, vt))
                    units.append(lambda p=p: emit_vones(p))
                    for q in range(4):
                        units.append(lambda p=p, q=q, vt=vt: emit_vtrans(p, q, vt))
                    return units

                def emit_st_exp(p, sc, e_t, tp):
                    # bf16 scores for t-tiles (2tp, 2tp+1), both heads, one
                    # s-chunk; e0/e1 use disjoint PE-array row halves.
                    ssl = bass.ts(sc, SC)
                    pst = [
                        st_psum.tile([128, 2, SC], F32, tag="st", name=f"pst{e}")
                        for e in range(2)
                    ]
                    for j in range(2):
                        tt = 2 * tp + j
                        for e in range(2):
                            esl = slice(e * 64, (e + 1) * 64)
                            nc.tensor.matmul(
                                pst[e][:, j, :],
                                kt_sb[p][esl, bass.ts(tt, 128)],
                                qt_sb[p][esl, ssl],
                                start=True,
                                stop=True,
                                tile_position=(e * 64, 0),
                            )
                    for e in range(2):
                        # exp tile split per chunk (GpSimd may not read PSUM,
                        # so only ACT/DVE can drain score psum banks).
                        # Pairs 0/1: ACT also carries proj bias-adds -> 8/8.
                        # Pair 2 has no proj work left -> ACT 9 / DVE 7.
                        if p == 2:
                            use_dve = tp in (1, 3, 5) or (tp == 7 and e == 0)
                        else:
                            use_dve = tp in DVE_TPS
                        if use_dve:
                            # Schraudolph bf16-bit exp on the DVE: the int16
                            # bit pattern round(s*A + B) IS bf16 exp(QK_SCALE*s)
                            # to ~2% per element; offloading these tiles keeps
                            # the ACT exp stream off the critical path.
                            nc.vector.tensor_scalar(
                                out=e_t[e][:, 2 * tp : 2 * tp + 2, :].bitcast(
                                    mybir.dt.int16),
                                in0=pst[e],
                                scalar1=EXP_A, scalar2=EXP_B,
                                op0=mybir.AluOpType.mult,
                                op1=mybir.AluOpType.add,
                            )
                        else:
                            nc.scalar.activation(
                                out=e_t[e][:, 2 * tp : 2 * tp + 2, :],
                                in_=pst[e],
                                func=mybir.ActivationFunctionType.Exp,
                                scale=QK_SCALE,
                            )

                def pv_units(p, sc, e_t):
                    # Flipped PV: stationary = P s-tile, moving = [V | 1].
                    # ctx[s, d] + denominator accumulate over all 16 t-tiles.
                    state = {}

                    def u_mm(e, qpair):
                        if qpair == 0:
                            # pair 2 has no projections left, so its e1 ctx
                            # can borrow the idle aux bank — e1's accumulation
                            # then no longer waits on e0's psum eviction.
                            pool = aux_psum if (e == 1 and p == 2) else ctx_psum
                            tag = "aux" if (e == 1 and p == 2) else "ctx"
                            state[e] = pool.tile(
                                [128, 4, 65], F32, tag=tag, name=f"pctx{e}")
                        pctx = state[e]
                        for q in (2 * qpair, 2 * qpair + 1):
                            for tt in range(NTT):
                                nc.tensor.matmul(
                                    pctx[:, q, :],
                                    e_t[e][:, tt, bass.ts(q, 128)],
                                    v_sb[p][:, tt, bass.ds(e * 65, 65)],
                                    start=(tt == 0),
                                    stop=(tt == NTT - 1),
                                )

                    def u_fin(e):
                        pctx = state[e]
                        cst = cspool.tile(
                            [128, 4, 65], F32, tag=f"cst{e}", name=f"cst{e}")
                        # psum eviction split across ACT (e0) and DVE (e1)
                        if e == 0:
                            nc.scalar.copy(out=cst, in_=pctx)
                        else:
                            nc.vector.tensor_copy(cst, pctx)
                        nc.sync.dma_start(
                            out=out.ap()[2 * p + e, :, 4 * sc : 4 * sc + 4, :],
                            in_=cst,
                        )

                    return [
                        lambda: u_mm(0, 0), lambda: u_mm(0, 1), lambda: u_fin(0),
                        lambda: u_mm(1, 0), lambda: u_mm(1, 1), lambda: u_fin(1),
                    ]

                # Software-pipelined emission: while each (pair, s-chunk)'s
                # ST+exp stream runs (exp-bound), the PE queue is fed pending
                # filler work - the previous chunk's PV matmuls and the next
                # pair's projections - so no engine head-of-line blocks.
                # Pair 0's own projections are interleaved with its first
                # s-chunk's scores so exp starts as early as possible.
                pending = []
                p0 = proj_units(0, skip_first=True)
                # p0 = [k1,q1,k2,q2,k3,q3, v0-v3, ones, vt0-vt3]
                # first x chunk lands in three pieces so the k-proj hc chain
                # starts as soon as hc0 arrives (DMA queues run concurrently)
                emit_x_load(0, 0, slice(0, 1))
                emit_x_load(0, 0, slice(1, 3))
                emit_x_load(0, 0, slice(3, 6))
                if not w_loaded[0]:
                    emit_w_load(0)
                emit_proj(0, 0, 1, kt_sb[0])
                emit_x_load(0, 1)
                if not w_loaded[0]:
                    emit_w_load(2)
                    w_loaded[0] = True
                emit_proj(0, 0, 0, qt_sb[0])
                # k(sc) must pop before the scores that read its t-range;
                # xa chunk sc must pop before k(sc)/q(sc).
                pending += [p0[0], lambda: emit_x_load(0, 2), p0[2],
                            lambda: emit_x_load(0, 3), p0[4],
                            p0[1], p0[3], p0[5]]     # k1,xa2,k2,xa3,k3,q1,q2,q3
                pending += p0[6:]                    # v-proj, ones, vtrans
                xb_units = [lambda sc=sc: emit_x_load(1, sc) for sc in range(NSC)]
                for p in range(NPAIR):
                    for sc in range(NSC):
                        e_t = [
                            epool.tile([128, NTT, SC], BF16, tag=f"e{e}", name=f"et{e}")
                            for e in range(2)
                        ]
                        ramp = p == 0 and sc < 2
                        # Pop pacing: 2/tp during ramp and pair boundaries
                        # (the proj(p+1) backlog incl. v-transposes must fully
                        # drain before pair p+1's pv is appended), but only
                        # 1 every other tp from p1sc2 on — pairs 0/1 are
                        # PE-bound while pair 2 has no proj filler left, so
                        # deferring pv one extra chunk (e_t bufs=3 allows it)
                        # smooths PE work into the pair-2 chunks.
                        if ramp or sc == NSC - 1:
                            pops = (2,) * 8
                        elif p == 2 or (p == 1 and sc >= 2):
                            pops = (0, 1) * 4
                        else:
                            pops = (1,) * 8
                        for tp in range(NTT // 2):
                            emit_st_exp(p, sc, e_t, tp)
                            for _ in range(pops[tp]):
                                if pending:
                                    pending.pop(0)()
                        # pv appended BEHIND any queued proj units: proj work
                        # is dependency-free PE filler, while pv(C) waits on
                        # chunk C's last exps — popping proj first fills the
                        # PE gaps.  e_t bufs=3 tolerates pv(C) landing in C+2.
                        if sc == 1 and p + 1 < NPAIR:
                            pending.extend(proj_units(p + 1))
                        pv = pv_units(p, sc, e_t)
                        pending.extend(pv)
                        if ramp and sc == 0:
                            pending.extend(xb_units)
                        # drain backlog so next pair's projections land on time
                        while len(pending) > 16:
                            pending.pop(0)()
                for ch in pending:
                    ch()

            for _rep in range(reps):
                _one_rep()

    nc.compile()
    return nc


_HALF_HEADS = {0: [0, 1, 2, 3, 4, 5], 1: [8, 9, 10, 11, 6, 7]}


def _prep_core_inputs(c, embeds, Wq, bq, Wk, bk, Wv, bv):
    b, half = divmod(c, 2)
    order = _HALF_HEADS[half]
    ga = 0 if half == 0 else 2
    bf = ml_dtypes.bfloat16

    def prep_x(e):
        # [S, H] -> [128, HC, S] with row h = hc*128 + p
        xt = np.ascontiguousarray(e.T).astype(bf)           # [H, S]
        return np.ascontiguousarray(
            xt.reshape(HC, 128, S).transpose(1, 0, 2))       # [128, HC, S]

    xa = prep_x(embeds[ga][b])
    xb = prep_x(embeds[1][b])

    wqkv = np.empty((H, 3, NPAIR, 128), np.float32)
    bias = np.empty((128, 3, NPAIR), np.float32)
    for p in range(NPAIR):
        h1, h2 = order[2 * p], order[2 * p + 1]
        for kind, W, bb, lam in (
            (0, Wq, bq, LAM), (1, Wk, bk, LAM), (2, Wv, bv, 1.0)
        ):
            wqkv[:, kind, p, 0:64] = W[h1] * lam
            wqkv[:, kind, p, 64:128] = W[h2] * lam
            bias[0:64, kind, p] = bb[h1] * lam
            bias[64:128, kind, p] = bb[h2] * lam
    # [H, 3, NPAIR, 128] -> [128(p), 3, NPAIR, HC, 128]
    wqkv = np.ascontiguousarray(
        wqkv.reshape(HC, 128, 3, NPAIR, 128).transpose(1, 2, 3, 0, 4)).astype(bf)

    return {"xa": xa, "xb": xb, "wqkv": wqkv, "bias": bias}


def kernel(embeds1, embeds2, embeds3, Wq, bq, Wk, bk, Wv, bv, _want_trace=False):
    if "nc" not in _CACHE:
        _CACHE["nc"] = _build_nc()
    nc = _CACHE["nc"]

    embeds = [np.asarray(embeds1), np.asarray(embeds2), np.asarray(embeds3)]
    Wq, bq = np.asarray(Wq), np.asarray(bq)
    Wk, bk = np.asarray(Wk), np.asarray(bk)
    Wv, bv = np.asarray(Wv), np.asarray(bv)

    in_maps = [
        _prep_core_inputs(c, embeds, Wq, bq, Wk, bk, Wv, bv) for c in range(8)
    ]
    res = run_bass_kernel_spmd(
        nc, in_maps, core_ids=list(range(8)), trace=_want_trace,
    )
    _CACHE["last_results"] = res

    full = np.empty((B, S, NH * D), np.float32)
    for c in range(8):
        b, half = divmod(c, 2)
        order = _HALF_HEADS[half]
        o = res.results[c]["out"]                   # [6, 128, NTT, 65]
        for j, h in enumerate(order):
            blk = o[j].transpose(1, 0, 2).reshape(S, 65)   # row s = tt*128+p
            full[b, :, h * 64 : (h + 1) * 64] = blk[:, 0:64] / blk[:, 64:65]
    return full
